# revision 2
# baseline (speedup 1.0000x reference)
"""MoE BasicBlock kernel v2 for TRN2.

W = sum_e alpha_e * w_e (21 experts), then conv3x3 -> BN -> relu -> conv3x3
-> BN -> +x -> relu on x [N,256,56,56] f32. Data-parallel across 8 cores
(4 images/core); the alpha-combine is oc-sharded (32 rows/core) and shared
via ONE fused bf16 AllGather per rep:

  - both weights' combined slices are cast to bf16 and gathered together
    (295KB/core -> 2.36MB), halving collective bytes 4x vs 2 f32 gathers
    and using one barrier instead of two;
  - no BN prescale on W2: the s2 scale is folded into the conv2 epilogue
    (STT computes s2*psum + x, ACT adds b2' inside the relu), so the bf16
    rounding happens exactly once, same as v1;
  - the conv2 residual reads the resident bf16 xpad tiles instead of
    re-loading x from DRAM (saves 12.8MB/rep/core and a dependency chain);
  - wpart/wgath are double-buffered by rep parity so rep r+1's combine and
    AllGather overlap rep r's convs (no WAR serialization).
"""

import numpy as np

import concourse.bass as bass
import concourse.mybir as mybir
import concourse.tile as tile
from concourse.masks import make_identity

FP32 = mybir.dt.float32
BF16 = mybir.dt.bfloat16

C = 256  # channels
CCH = 2  # channel chunks of 128
H = W = 56
HP = WP = 58  # padded
E = 21  # experts
ROWT = 8  # output rows per psum tile
NRT = H // ROWT  # 7 row tiles
NTILE = ROWT * W  # 448
KHW = 9  # kernel positions
IC9 = C * KHW  # 2304, per-oc-row weight elements
EPS = 1e-5


def split_multi_waits(nc):
    """The installed walrus accepts at most one sync-wait per instruction
    (two for EventSemaphore). Tile's sem assignment can emit more; split the
    extras onto injected same-engine nops placed immediately before the
    offending instruction (equivalent semantics for in-order engine streams).
    """
    n_split = 0
    n_dma_split = 0
    for bb in nc.main_func.blocks:
        new_list = []
        for inst in list(bb.instructions):
            si = inst.sync_info
            waits = list(si.on_wait) if si is not None and si.on_wait else []
            cap = 2 if isinstance(inst, mybir.InstEventSemaphore) else 1
            if len(waits) > cap:
                if getattr(inst, "queue", None) is not None:
                    n_dma_split += 1
                extra, keep = waits[:-cap], waits[-cap:]
                for w in extra:
                    nop = nc.engines[inst.engine].nop(hint="waitsplit", nofuse=True)
                    # nop() appended itself to nc.cur_bb; pull it back out.
                    host_bb = nc.cur_bb
                    assert host_bb.bb.instructions[-1] is nop.ins
                    host_bb.bb.instructions.pop()
                    nop.ins.sync_info = mybir.SyncInfo(on_update=[], on_wait=[w])
                    new_list.append(nop.ins)
                    n_split += 1
                inst.sync_info = mybir.SyncInfo(
                    on_update=list(si.on_update) if si.on_update else [], on_wait=keep
                )
            new_list.append(inst)
        bb.instructions[:] = new_list
    return n_split, n_dma_split


def build_nc(npc=4, repeat=1, n_cores=8):
    """Per-core Bass program. npc = images per core (4). repeat>1 re-emits
    the computation (for differential wall-clock timing)."""
    nc = bass.Bass(
        "TRN2", target_bir_lowering=False, debug=False, num_devices=n_cores
    )

    OSH = C // n_cores  # oc rows combined per core (32)
    SFREE = OSH * IC9 // 128  # 576: per-partition cols of a core's slice

    x = nc.dram_tensor("x", [npc, C, H, W], FP32, kind="ExternalInput")
    alpha = nc.dram_tensor("alpha", [E], FP32, kind="ExternalInput")
    w1 = nc.dram_tensor("w1s", [E, OSH, C, 3, 3], FP32, kind="ExternalInput")
    w2 = nc.dram_tensor("w2s", [E, OSH, C, 3, 3], FP32, kind="ExternalInput")
    bn = {}
    for nm in ("g1", "b1", "m1", "v1", "g2", "b2", "m2", "v2"):
        bn[nm] = nc.dram_tensor(nm, [C], FP32, kind="ExternalInput")
    out = nc.dram_tensor("out", [npc, C, H, W], FP32, kind="ExternalOutput")

    xap = x.ap().rearrange("n c h w -> n c (h w)")
    oap = out.ap().rearrange("n c h w -> n c (h w)")
    # per-core slice [E, OSH, IC9] flattened to [128, SFREE] per expert
    w1ap = w1.ap().rearrange("e o i h w -> e (o i h w)")
    w2ap = w2.ap().rearrange("e o i h w -> e (o i h w)")
    # double-buffered collective staging (rep parity)
    wparts = [nc.dram_tensor(f"wpart{i}", [2 * OSH * IC9], BF16) for i in range(2)]
    # gathered layout: [core, wi, OSH, IC9] (AllGather rank-concat order)
    wgaths = [
        nc.dram_tensor(f"wgath{i}", [n_cores, 2, OSH, IC9], BF16, addr_space="Shared")
        for i in range(2)
    ]

    with tile.TileContext(nc) as tc:
        import contextlib

        with contextlib.ExitStack() as ctx:
            singles = ctx.enter_context(tc.tile_pool(name="singles", bufs=1))
            epool = ctx.enter_context(tc.tile_pool(name="epool", bufs=3))
            wfpool = ctx.enter_context(tc.tile_pool(name="wfpool", bufs=2))
            accpool = ctx.enter_context(tc.tile_pool(name="accpool", bufs=2))
            xspool = ctx.enter_context(tc.tile_pool(name="xspool", bufs=2))
            xpads = ctx.enter_context(tc.tile_pool(name="xpads", bufs=2 * npc))
            ypads = ctx.enter_context(tc.tile_pool(name="ypads", bufs=6))
            obpool = ctx.enter_context(tc.tile_pool(name="obpool", bufs=4))
            cpsum = ctx.enter_context(tc.tile_pool(name="cpsum", bufs=6, space="PSUM"))
            tpsum = ctx.enter_context(tc.tile_pool(name="tpsum", bufs=2, space="PSUM"))

            # ---- stage 0: BN params, alpha, identity ----
            ident = singles.tile([128, 128], BF16, tag="ident")
            make_identity(nc, ident[:])

            zero_c = singles.tile([128, 1], FP32, tag="zero_c")
            nc.vector.memset(zero_c[:], 0.0)
            nc.const_aps.aps[(FP32, 0.0)] = zero_c[:]
            eps_c = singles.tile([128, 1], FP32, tag="eps_c")
            nc.vector.memset(eps_c[:], EPS)

            alpha_sb = singles.tile([128, E], FP32, tag="alpha")
            nc.sync.dma_start(
                out=alpha_sb[:],
                in_=bass.AP(tensor=alpha.ap().tensor, offset=0, ap=[[0, 128], [1, E]]),
            )

            bns = {}
            for nm in ("g1", "b1", "m1", "v1", "g2", "b2", "m2", "v2"):
                t = singles.tile([128, CCH], FP32, name=f"bn_{nm}", tag=f"bn_{nm}")
                nc.sync.dma_start(
                    out=t[:],
                    in_=bass.AP(
                        tensor=bn[nm].ap().tensor, offset=0, ap=[[1, 128], [128, CCH]]
                    ),
                )
                bns[nm] = t

            def bn_fold(g, b, m, v, idx):
                # s = g / sqrt(v + eps) (one Newton step on ACT sqrt),
                # b' = b - m * s
                sq = singles.tile([128, CCH], FP32, name=f"bn_sq{idx}", tag=f"bn_sq{idx}")
                nc.scalar.activation(
                    sq[:], v[:], mybir.ActivationFunctionType.Sqrt, bias=eps_c[:]
                )
                r = singles.tile([128, CCH], FP32, name=f"bn_r{idx}", tag=f"bn_r{idx}")
                nc.vector.reciprocal(r[:], sq[:])
                ve = singles.tile([128, CCH], FP32, name=f"bn_ve{idx}", tag=f"bn_ve{idx}")
                nc.vector.tensor_scalar_add(ve[:], v[:], EPS)
                t1 = singles.tile([128, CCH], FP32, name=f"bn_t1{idx}", tag=f"bn_t1{idx}")
                nc.vector.tensor_mul(t1[:], ve[:], r[:])
                nc.vector.tensor_add(t1[:], t1[:], sq[:])
                nc.vector.tensor_scalar_mul(t1[:], t1[:], 0.5)  # refined sqrt
                nc.vector.reciprocal(r[:], t1[:])  # refined rsqrt
                s = singles.tile([128, CCH], FP32, name=f"bn_s{idx}", tag=f"bn_s{idx}")
                nc.vector.tensor_mul(s[:], g[:], r[:])
                bp = singles.tile([128, CCH], FP32, name=f"bn_bp{idx}", tag=f"bn_bp{idx}")
                nc.vector.tensor_mul(bp[:], m[:], s[:])
                nc.vector.tensor_sub(bp[:], b[:], bp[:])
                return s, bp

            s1, b1p = bn_fold(bns["g1"], bns["b1"], bns["m1"], bns["v1"], 1)
            s2, b2p = bn_fold(bns["g2"], bns["b2"], bns["m2"], bns["v2"], 2)

            # lhsT[wi][ic][oc]: [128(ic), 9, 128(oc)] bf16
            lhsT = [
                [
                    [
                        singles.tile(
                            [128, KHW, 128], BF16,
                            name=f"lhsT_{wi}_{ic}_{oc}", tag=f"lhsT_{wi}_{ic}_{oc}",
                        )
                        for oc in range(CCH)
                    ]
                    for ic in range(CCH)
                ]
                for wi in range(2)
            ]

            # ---- sharded combine + one fused bf16 AllGather ----
            def combine_gather(par):
                accb = accpool.tile([128, 2 * SFREE], BF16, name="accb", tag="accb")
                for wi, wap in ((0, w1ap), (1, w2ap)):
                    acc = wfpool.tile([128, SFREE], FP32, name="sacc", tag="sacc")
                    for e in range(E):
                        est = epool.tile([128, SFREE], FP32, name="sest", tag="sest")
                        nc.sync.dma_start(
                            out=est[:],
                            in_=wap[e].rearrange("(p f) -> p f", p=128),
                        )
                        if e == 0:
                            nc.vector.tensor_scalar_mul(
                                acc[:], est[:], alpha_sb[:, 0:1]
                            )
                        else:
                            nc.vector.scalar_tensor_tensor(
                                acc[:], est[:], alpha_sb[:, e : e + 1], acc[:],
                                op0=mybir.AluOpType.mult, op1=mybir.AluOpType.add,
                            )
                    # bf16 cast on ACT (keeps DVE free for the next chain)
                    nc.scalar.copy(
                        out=accb[:, wi * SFREE : (wi + 1) * SFREE], in_=acc[:]
                    )
                # de-interleave: accb cols [wi*SFREE:(wi+1)*SFREE] -> flat
                # [wi, OSH, IC9] so the gathered layout is [c, wi, OSH, IC9]
                nc.sync.dma_start(
                    out=wparts[par].ap().rearrange("(w p f) -> p w f", w=2, p=128),
                    in_=accb[:].rearrange("p (w f) -> p w f", w=2),
                )
                nc.gpsimd.collective_compute(
                    "AllGather",
                    mybir.AluOpType.bypass,
                    replica_groups=[list(range(n_cores))],
                    ins=[wparts[par].ap().opt()],
                    outs=[wgaths[par].ap().rearrange("c w o f -> (c w o f)").opt()],
                )

            def load_transpose(par, wi):
                # gathered rows of weight wi live at wgath[4*oc+k, wi, :, :]
                for oc in range(CCH):
                    wf = wfpool.tile([128, IC9], BF16, name="wfull", tag="wfull")
                    for k in range(4):
                        nc.sync.dma_start(
                            out=wf[32 * k : 32 * (k + 1), :],
                            in_=wgaths[par].ap()[4 * oc + k, wi],
                        )
                    wfr = wf[:].rearrange("p (c i r) -> p c i r", c=CCH, r=KHW)
                    for ic in range(CCH):
                        for pos in range(KHW):
                            pt = tpsum.tile([128, 128], BF16, name="tpsum", tag="tpsum")
                            nc.tensor.transpose(pt[:], wfr[:, ic, :, pos], ident[:])
                            nc.vector.tensor_copy(lhsT[wi][ic][oc][:, pos, :], pt[:])

            # ---- xpad fill ----
            def fill_xpad(n):
                tiles = []
                for c in range(CCH):
                    xst = xspool.tile([128, H * W], FP32, name="xstage", tag="xstage")
                    nc.sync.dma_start(
                        out=xst[:], in_=xap[n, c * 128 : (c + 1) * 128, :]
                    )
                    xp = xpads.tile([128, HP * WP], BF16, name="xpad", tag="xpad")
                    xpr = xp[:].rearrange("p (r c) -> p r c", r=HP)
                    zero_ring(xpr)
                    nc.scalar.copy(
                        out=xpr[:, 1 : H + 1, 1 : W + 1],
                        in_=xst[:].rearrange("p (r c) -> p r c", r=H),
                    )
                    tiles.append(xp)
                return tiles

            def zero_ring(tr):
                nc.vector.memset(tr[:, 0, :], 0.0)
                nc.vector.memset(tr[:, HP - 1, :], 0.0)
                nc.vector.memset(tr[:, 1 : HP - 1, 0:1], 0.0)
                nc.vector.memset(tr[:, 1 : HP - 1, WP - 1 : WP], 0.0)

            def conv(n, src_tiles, wi, dst):
                """One conv3x3 over image n. src_tiles: [128,HP*WP] bf16 per ic
                chunk. dst: ypads tiles (wi=0); wi=1 adds residual from
                src-of-conv1 xpads and writes DRAM."""
                srcv = [
                    t[:].rearrange("p (r c) -> p r c", r=HP) for t in src_tiles
                ]
                if wi == 1:
                    resv = [
                        t[:].rearrange("p (r c) -> p r c", r=HP) for t in dst
                    ]
                for oc in range(CCH):
                    for rt in range(NRT):
                        ps = cpsum.tile([128, NTILE], FP32, name="cpsum", tag="cpsum")
                        k = 0
                        for ic in range(CCH):
                            for ky in range(3):
                                for kx in range(3):
                                    rhs = srcv[ic][
                                        :, rt * ROWT + ky : rt * ROWT + ky + ROWT,
                                        kx : kx + W,
                                    ]
                                    nc.tensor.matmul(
                                        ps[:],
                                        lhsT[wi][ic][oc][:, 3 * ky + kx, :],
                                        rhs,
                                        start=(k == 0),
                                        stop=(k == 17),
                                    )
                                    k += 1
                        psr = ps[:].rearrange("p (r c) -> p r c", r=ROWT)
                        if wi == 0:
                            ypr = dst[oc][:].rearrange("p (r c) -> p r c", r=HP)
                            nc.scalar.activation(
                                ypr[:, rt * ROWT + 1 : rt * ROWT + 1 + ROWT, 1 : W + 1],
                                psr,
                                mybir.ActivationFunctionType.Relu,
                                bias=b1p[:, oc : oc + 1],
                                scale=s1[:, oc : oc + 1],
                            )
                        else:
                            ob = obpool.tile([128, NTILE], FP32, name="ob", tag="ob")
                            obr = ob[:].rearrange("p (r c) -> p r c", r=ROWT)
                            # ob = s2*psum + x  (residual from resident xpad)
                            nc.vector.scalar_tensor_tensor(
                                obr,
                                psr,
                                s2[:, oc : oc + 1],
                                resv[oc][
                                    :, rt * ROWT + 1 : rt * ROWT + 1 + ROWT, 1 : W + 1
                                ],
                                op0=mybir.AluOpType.mult,
                                op1=mybir.AluOpType.add,
                            )
                            # out = relu(ob + b2')
                            nc.scalar.activation(
                                ob[:], ob[:], mybir.ActivationFunctionType.Relu,
                                bias=b2p[:, oc : oc + 1],
                            )
                            nc.sync.dma_start(
                                out=oap[
                                    n,
                                    oc * 128 : (oc + 1) * 128,
                                    rt * NTILE : (rt + 1) * NTILE,
                                ],
                                in_=ob[:],
                            )

            def alloc_ypad():
                tiles = []
                for c in range(CCH):
                    yp = ypads.tile([128, HP * WP], BF16, name="ypad", tag="ypad")
                    zero_ring(yp[:].rearrange("p (r c) -> p r c", r=HP))
                    tiles.append(yp)
                return tiles

            # ---- emission schedule (engine streams are in-order) ----
            # Software-pipelined: rep r+1's combine+AllGather is emitted
            # during rep r's conv1 phase (DVE/gpsimd are idle there), so the
            # collective runs under rep r's conv matmuls and wgath[par'] is
            # ready well before rep r+1 starts.
            assert npc == 4
            combine_gather(0)
            for rep in range(repeat):
                par = rep % 2
                load_transpose(par, 0)
                xps = {0: fill_xpad(0), 1: fill_xpad(1)}
                yps = {}
                yps[0] = alloc_ypad()
                conv(0, xps[0], 0, yps[0])
                xps[2] = fill_xpad(2)
                if rep + 1 < repeat:
                    combine_gather(1 - par)
                load_transpose(par, 1)
                yps[1] = alloc_ypad()
                conv(1, xps[1], 0, yps[1])
                xps[3] = fill_xpad(3)
                yps[2] = alloc_ypad()
                conv(2, xps[2], 0, yps[2])
                conv(0, yps[0], 1, xps[0])
                yps[3] = alloc_ypad()
                conv(3, xps[3], 0, yps[3])
                for n in range(1, npc):
                    conv(n, yps[n], 1, xps[n])

    n_split, n_dma_split = split_multi_waits(nc)
    return nc, (n_split, n_dma_split)


# ---------------------------------------------------------------------------
# Host-side entry point: takes FULL inputs, shards batch across 8 cores.
# ---------------------------------------------------------------------------
_NC_CACHE = {}


def kernel(**inputs):
    from concourse.bass_utils import run_bass_kernel_spmd

    x = np.ascontiguousarray(np.asarray(inputs["x"], dtype=np.float32))
    n_total = x.shape[0]
    n_cores = 8
    npc = n_total // n_cores
    assert npc * n_cores == n_total

    key = npc
    if key not in _NC_CACHE:
        _NC_CACHE[key] = build_nc(npc=npc)[0]
    nc = _NC_CACHE[key]

    w1 = np.asarray(inputs["w1"], dtype=np.float32)
    w2 = np.asarray(inputs["w2"], dtype=np.float32)
    osh = w1.shape[1] // n_cores
    shared = {
        k: np.ascontiguousarray(np.asarray(v, dtype=np.float32))
        for k, v in inputs.items()
        if k not in ("x", "w1", "w2")
    }
    in_maps = [
        {
            "x": x[c * npc : (c + 1) * npc],
            "w1s": np.ascontiguousarray(w1[:, c * osh : (c + 1) * osh]),
            "w2s": np.ascontiguousarray(w2[:, c * osh : (c + 1) * osh]),
            **shared,
        }
        for c in range(n_cores)
    ]
    res = run_bass_kernel_spmd(nc, in_maps, core_ids=list(range(n_cores)))
    return np.concatenate([res.results[c]["out"] for c in range(n_cores)], axis=0)
